# revision 8
# baseline (speedup 1.0000x reference)
"""Trainium2 Bass kernel for MultiLayerJKNet (6-layer GraphConv + JK-concat).

Strategy: nodes sharded across 8 cores (padded 10000 -> 10240, 1280/core).
Per layer: data-parallel GEMM (h @ W with BN scale folded into W), AllGather
of the GEMM outputs, then per-core segment-sum aggregation done as
indirect-DMA row gather + one-hot matmuls on the TensorEngine (one-hot
weights carry the ns[src]*nd[dst] normalization; BN shift enters via a
rank-1 matmul; ReLU on ScalarE). JK-concat final GEMM is accumulated
per-layer into SBUF. All fp32.
"""
import numpy as np

N = 10000
E = 160000
H = 512
L = 6
C = 64
EPS = 1e-5
NCORES = 8
P = 128
N_PAD = 10240
NPC = N_PAD // NCORES          # 1280 nodes per core
MB = NPC // P                  # 10 m-tiles (dst blocks) per core
KC = H // P                    # 4 contraction chunks

_cache = {}


def _host_prep(x, W0, Ws, bs, gamma, beta, run_mean, run_var, linW, linb, src, dst):
    """Fold BN into weights, partition edges per core/block, build one-hot
    segment matrices and gather indices."""
    src = np.asarray(src).astype(np.int64)
    dst = np.asarray(dst).astype(np.int64)
    x = np.asarray(x, dtype=np.float32)

    deg_out = np.bincount(src, minlength=N).astype(np.float32)
    deg_in = np.bincount(dst, minlength=N).astype(np.float32)
    ns = np.clip(deg_out, 1.0, None) ** -0.5
    nd = np.clip(deg_in, 1.0, None) ** -0.5

    scale = (gamma * (1.0 / np.sqrt(run_var + EPS))).astype(np.float32)   # [L,H]
    shift = ((bs - run_mean) * scale + beta).astype(np.float32)           # [L,H]
    Wp = np.empty((L, H, H), np.float32)
    Wp[0] = W0 * scale[0][None, :]
    for i in range(1, L):
        Wp[i] = Ws[i - 1] * scale[i][None, :]
    linW = np.asarray(linW, dtype=np.float32)   # [L*H, C]

    # --- edge partitioning ---
    # per (core, dst-block of 128): list of edge ids
    blk_of_dst = dst // P                      # 0 .. 79 (globally)
    order = np.argsort(blk_of_dst, kind="stable")
    sorted_e = order
    sorted_blk = blk_of_dst[order]
    # boundaries per global block
    counts = np.bincount(sorted_blk, minlength=N_PAD // P)
    S = int(np.max((counts + P - 1) // P))     # chunks per block (same all cores)
    starts = np.zeros(N_PAD // P, np.int64)
    starts[1:] = np.cumsum(counts)[:-1]

    KB = S * P                                  # padded edges per block
    idx_all = np.zeros((NCORES, MB, P, S), np.int32)      # gather indices [p, s]
    oh_all = np.zeros((NCORES, MB, S, P, P), np.float32)  # one-hot lhsT per chunk
    ew = (ns[src] * nd[dst]).astype(np.float32)
    for c in range(NCORES):
        for b in range(MB):
            g = c * MB + b                      # global block id
            n_real = counts[g]
            el = sorted_e[starts[g]:starts[g] + n_real]
            # edge j -> (chunk s=j//P, lane p=j%P)
            s_arr = np.arange(n_real) // P
            p_arr = np.arange(n_real) % P
            idx_all[c, b, p_arr, s_arr] = src[el].astype(np.int32)
            d_local = (dst[el] - g * P).astype(np.int64)   # 0..127
            oh_all[c, b, s_arr, p_arr, d_local] = ew[el]

    x_pad = np.zeros((N_PAD, H), np.float32)
    x_pad[:N] = x

    per_core = []
    # w_all layout: [128, L*KC*H]; slice (i,k) at [:, (i*KC+k)*H : +H] equals
    # Wp[i][k*128:(k+1)*128, :]
    w_all = np.zeros((P, L * KC * H), np.float32)
    for i in range(L):
        for k in range(KC):
            w_all[:, (i * KC + k) * H:(i * KC + k + 1) * H] = Wp[i][k * P:(k + 1) * P, :]
    lw_all = np.zeros((P, L * KC * C), np.float32)
    for i in range(L):
        for k in range(KC):
            lw_all[:, (i * KC + k) * C:(i * KC + k + 1) * C] = \
                linW[i * H + k * P: i * H + (k + 1) * P, :]
    shift_row = shift.reshape(1, L * H).astype(np.float32)  # [1, L*H]
    ones_row = np.ones((1, P), np.float32)

    for c in range(NCORES):
        xT = np.ascontiguousarray(x_pad[c * NPC:(c + 1) * NPC].T)   # [H, NPC]
        per_core.append({
            "xT": xT,
            "w_all": w_all,
            "lw_all": lw_all,
            "shift_row": shift_row,
            "ones_row": ones_row,
            "idx": np.ascontiguousarray(idx_all[c].transpose(1, 0, 2)
                                        .reshape(P, MB * S)),   # [128, MB*S]
            # onehot DRAM layout per block: [P, S*P] with chunk s at
            # [:, s*P:(s+1)*P] equal to oh[b, s] (lhsT [lane, dstlocal])
            "onehot": np.ascontiguousarray(
                oh_all[c].transpose(0, 2, 1, 3).reshape(MB, P, S * P)
                .transpose(1, 0, 2).reshape(P, MB * S * P)),
        })
    meta = {"S": S, "ns": ns, "nd": nd, "shift": shift, "Wp": Wp,
            "linW": linW, "linb": np.asarray(linb, np.float32),
            "x_pad": x_pad, "idx_all": idx_all, "oh_all": oh_all}
    return per_core, meta


def _numpy_model(per_core, meta):
    """Exact numpy mirror of the device algorithm (for validation)."""
    S = meta["S"]
    h = meta["x_pad"].copy()
    out = np.zeros((N_PAD, C), np.float32)
    for i in range(L):
        hw = h @ meta["Wp"][i]
        h_next = np.zeros_like(h)
        for c in range(NCORES):
            for b in range(MB):
                ps = np.zeros((P, H), np.float32)
                for s in range(S):
                    oh = meta["oh_all"][c, b, s]
                    g = hw[meta["idx_all"][c, b, :, s]]
                    ps += oh.T @ g
                ps += meta["shift"][i][None, :]
                h_next[(c * MB + b) * P:(c * MB + b + 1) * P] = np.maximum(ps, 0)
        h = h_next
        out += h @ meta["linW"][i * H:(i + 1) * H, :]
    out += meta["linb"][None, :]
    return out[:N]


def _build_bass(S):
    import concourse.bacc as bacc
    import concourse.bass as bass
    import concourse.mybir as mybir
    import concourse.tile as tile
    from concourse.masks import make_identity

    f32 = mybir.dt.float32
    nc = bacc.Bacc("TRN2", target_bir_lowering=False, debug=False,
                   enable_asserts=False, num_devices=NCORES)

    xT_d = nc.dram_tensor("xT", [H, NPC], f32, kind="ExternalInput").ap()
    w_d = nc.dram_tensor("w_all", [P, L * KC * H], f32, kind="ExternalInput").ap()
    lw_d = nc.dram_tensor("lw_all", [P, L * KC * C], f32, kind="ExternalInput").ap()
    shift_d = nc.dram_tensor("shift_row", [1, L * H], f32, kind="ExternalInput").ap()
    ones_d = nc.dram_tensor("ones_row", [1, P], f32, kind="ExternalInput").ap()
    idx_d = nc.dram_tensor("idx", [P, MB * S], mybir.dt.int32, kind="ExternalInput").ap()
    oh_d = nc.dram_tensor("onehot", [P, MB * S * P], f32, kind="ExternalInput").ap()
    out_d = nc.dram_tensor("out", [NPC, C], f32, kind="ExternalOutput").ap()

    with tile.TileContext(nc) as tc:
        with (
            tc.tile_pool(name="persist", bufs=1) as pp,
            tc.tile_pool(name="hw", bufs=3) as hwp,
            tc.tile_pool(name="gath", bufs=8) as gp,
            tc.tile_pool(name="oh", bufs=2) as ohp,
            tc.tile_pool(name="hb", bufs=2) as hbp,
            tc.tile_pool(name="psg", bufs=2, space="PSUM") as psg,
            tc.tile_pool(name="pso", bufs=2, space="PSUM") as pso,
            tc.tile_pool(name="psa", bufs=2, space="PSUM") as psa,
            tc.tile_pool(name="pst", bufs=2, space="PSUM") as pst,
            tc.tile_pool(name="dram", bufs=2, space="DRAM") as dp,
        ):
            # persistent tiles
            hT = pp.tile([P, KC * NPC], f32)          # h transposed, chunk k at [:, k*NPC:+NPC]
            w_all = pp.tile([P, L * KC * H], f32)
            lw_all = pp.tile([P, L * KC * C], f32)
            out_acc = pp.tile([P, MB * C], f32)
            idx_t = pp.tile([P, MB * S], mybir.dt.int32)
            shift_t = pp.tile([1, L * H], f32)
            ones_t = pp.tile([1, P], f32)
            ident = pp.tile([P, P], f32)

            make_identity(nc, ident[:])
            nc.sync.dma_start(out=w_all[:], in_=w_d[:])
            nc.sync.dma_start(out=lw_all[:], in_=lw_d[:])
            nc.sync.dma_start(out=idx_t[:], in_=idx_d[:])
            nc.sync.dma_start(out=shift_t[:], in_=shift_d[:])
            nc.sync.dma_start(out=ones_t[:], in_=ones_d[:])
            # load x^T into hT
            for k in range(KC):
                nc.sync.dma_start(out=hT[:, k * NPC:(k + 1) * NPC],
                                  in_=xT_d[k * P:(k + 1) * P, :])
            nc.gpsimd.memset(out_acc[:], 0.0)

            def out_gemm(i):
                # out_acc += hs[i]^T.T @ linW_i   (hT currently holds hs[i]^T)
                for m in range(MB):
                    po = pso.tile([P, C], f32, space="PSUM")
                    for k in range(KC):
                        nc.tensor.matmul(
                            out=po[:],
                            lhsT=hT[:, k * NPC + m * P: k * NPC + (m + 1) * P],
                            rhs=lw_all[:, (i * KC + k) * C:(i * KC + k + 1) * C],
                            start=(k == 0), stop=(k == KC - 1))
                    nc.vector.tensor_add(out=out_acc[:, m * C:(m + 1) * C],
                                         in0=out_acc[:, m * C:(m + 1) * C],
                                         in1=po[:])

            for i in range(L):
                cc_in = dp.tile([NPC, H], f32)
                cc_out = dp.tile([N_PAD, H], f32)
                # ---- step 1: GEMM hw = h @ W'_i ----
                for m in range(MB):
                    ps = psg.tile([P, H], f32, space="PSUM")
                    for k in range(KC):
                        nc.tensor.matmul(
                            out=ps[:],
                            lhsT=hT[:, k * NPC + m * P: k * NPC + (m + 1) * P],
                            rhs=w_all[:, (i * KC + k) * H:(i * KC + k + 1) * H],
                            start=(k == 0), stop=(k == KC - 1))
                    hwm = hwp.tile([P, H], f32)
                    nc.vector.tensor_copy(out=hwm[:], in_=ps[:])
                    nc.sync.dma_start(out=cc_in[m * P:(m + 1) * P, :], in_=hwm[:])
                # out-GEMM for previous layer's h (hT still holds hs[i-1]^T;
                # for i==0 hT holds x which does NOT contribute)
                if i > 0:
                    out_gemm(i - 1)
                # ---- step 2: AllGather ----
                nc.gpsimd.collective_compute(
                    "AllGather", mybir.AluOpType.bypass,
                    replica_groups=[list(range(NCORES))],
                    ins=[cc_in.opt()], outs=[cc_out.opt()])
                # ---- step 3: aggregation per dst block ----
                for b in range(MB):
                    oh = ohp.tile([P, S * P], f32)
                    nc.sync.dma_start(out=oh[:],
                                      in_=oh_d[:, b * S * P:(b + 1) * S * P])
                    pa = psa.tile([P, H], f32, space="PSUM")
                    for s in range(S):
                        g = gp.tile([P, H], f32)
                        nc.gpsimd.indirect_dma_start(
                            out=g[:], out_offset=None, in_=cc_out[:],
                            in_offset=bass.IndirectOffsetOnAxis(
                                ap=idx_t[:, b * S + s:b * S + s + 1], axis=0))
                        nc.tensor.matmul(
                            out=pa[:],
                            lhsT=oh[:, s * P:(s + 1) * P],
                            rhs=g[:],
                            start=(s == 0), stop=False)
                    nc.tensor.matmul(
                        out=pa[:], lhsT=ones_t[:1, :],
                        rhs=shift_t[:1, i * H:(i + 1) * H],
                        start=False, stop=True)
                    hb = hbp.tile([P, H], f32)
                    nc.scalar.activation(out=hb[:], in_=pa[:],
                                         func=mybir.ActivationFunctionType.Relu)
                    # transpose back into hT
                    for k in range(KC):
                        pt = pst.tile([P, P], f32, space="PSUM")
                        nc.tensor.transpose(out=pt[:],
                                            in_=hb[:, k * P:(k + 1) * P],
                                            identity=ident[:])
                        nc.vector.tensor_copy(
                            out=hT[:, k * NPC + b * P: k * NPC + (b + 1) * P],
                            in_=pt[:])
            # final layer's JK contribution
            out_gemm(L - 1)
            # write out: out_d view [128, MB, C] <- out_acc [128, MB*C]
            out_view = out_d.rearrange("(m p) c -> p m c", p=P)
            nc.sync.dma_start(out=out_view, in_=out_acc[:].rearrange("p (m c) -> p m c", c=C))
    nc.compile()
    return nc


def kernel(**inputs):
    per_core, meta = _host_prep(**inputs)
    S = meta["S"]
    key = ("nc", S)
    if key not in _cache:
        _cache[key] = _build_bass(S)
    nc = _cache[key]
    from concourse.bass_utils import run_bass_kernel_spmd
    in_maps = [{k: v for k, v in pc.items()} for pc in per_core]
    res = run_bass_kernel_spmd(nc, in_maps, core_ids=list(range(NCORES)))
    out = np.concatenate([res.results[c]["out"] for c in range(NCORES)], axis=0)
    out = out[:N] + meta["linb"][None, :]
    return out.astype(np.float32)
